# revision 32
# baseline (speedup 1.0000x reference)
"""Trainium2 Bass kernel for causal softmax-free multi-head attention (retention).

Reference computation (per batch b):
    kqv = x @ W1 + b1 ; k, q, v = split(kqv, 3)   [split order k, q, v]
    per head h (dh = 64):  attn = tril(q_h @ k_h^T) ; o_h = attn @ v_h
    out = concat_h(o_h) @ W2 + b2

Sharding: 8 cores = 2 batches x 4 head-groups (4 heads each). Each core
computes its batch's projections restricted to its heads' weight columns,
the attention for its 4 heads, and a partial output projection
(out_local @ W2[rows of its heads]). Host sums the 4 partials per batch.

Algorithm: chunked linear attention. tril(QK^T)V is computed per 256-token
block as  O = Q @ S + tril_block(Q K_blk^T) V_blk, with the running state
S = K^T V accumulated over previous blocks ([64,64] per head).

All matmul operands are bf16 (rel err ~6e-3, gate is 2e-2); PSUM stays f32.

v2 layout strategy:
  - Q^T, K^T, V^T all computed feature-major in ap=512 waves from
    single-DMA weight tiles (w1 packed [1024, 768] K|V|Q on host).
  - Token-major K, V (needed for the state update / attn.V contractions)
    come from hardware DMA transposes (XBAR) of K^T/V^T: [128,2048] ->
    [128,16,128] tiled, 2 triggers per tensor, zero PE cost.
  - Scores contract K=64 directly (bf16 allows K<128) with partition-offset
    operands - no zero-padded K^T copies.
  - No SBUF pool aliasing (everything fits in ~12MB) so W2/masks/zeros
    load up front and phase transitions don't stall on WAR deps.
"""

import numpy as np

import concourse.bacc as bacc
import concourse.mybir as mybir
import concourse.tile as tile
from concourse.bass_utils import run_bass_kernel_spmd

F32 = mybir.dt.float32
BF16 = mybir.dt.bfloat16
AF = mybir.ActivationFunctionType

B, T, D = 2, 2048, 1024
H, DH = 16, 64
HPC = 4           # heads per core
FH = HPC * DH     # 256 features per core per tensor
BLK = 256         # state-update block (2 x 128-token chunks)
NBLK = T // BLK   # 8
ND = D // 128     # 8 contraction chunks
NQT = T // 512    # 4 wide token tiles

TRACE = False
TRACE_DIR = None
LAST_RESULTS = [None]


def _build():
    nc = bacc.Bacc("TRN2", target_bir_lowering=False, debug=False, num_devices=8)

    xT = nc.dram_tensor("xT", [D, T], BF16, kind="ExternalInput").ap()
    w1 = nc.dram_tensor("w1", [D, 3 * FH], BF16, kind="ExternalInput").ap()
    w2 = nc.dram_tensor("w2", [FH, D], BF16, kind="ExternalInput").ap()
    b1p = nc.dram_tensor("b1p", [128, 6], F32, kind="ExternalInput").ap()
    masks = nc.dram_tensor("masks", [128, 512], F32, kind="ExternalInput").ap()
    # output chunk-major: out[p, dc, t] = full_out[dc*128 + p, t]; host reassembles
    out = nc.dram_tensor("out", [128, ND * T], BF16, kind="ExternalOutput").ap()
    out3 = out.rearrange("p (c t) -> p c t", c=ND)

    with tile.TileContext(nc) as tc:
        with (
            tc.tile_pool(name="persist", bufs=1) as pp,
            tc.tile_pool(name="work", bufs=4) as wp,
            tc.tile_pool(name="psA", bufs=4, space="PSUM") as psA,
            tc.tile_pool(name="psO", bufs=2, space="PSUM") as psO,
            tc.tile_pool(name="psU", bufs=2, space="PSUM") as psU,
        ):
            # ---- persistent SBUF tiles -------------------------------------
            xt = [pp.tile([128, T], BF16, name=f"xt{i}", tag=f"xt{i}") for i in range(ND)]
            w1d = [pp.tile([128, 3 * FH], BF16, name=f"w1d{i}", tag=f"w1d{i}")
                   for i in range(ND)]
            w2_sb = pp.tile([128, 2, D], BF16, name="w2_sb", tag="w2_sb")
            b1_sb = pp.tile([128, 6], F32, name="b1_sb", tag="b1_sb")
            mk_sb = pp.tile([128, 512], F32, name="mk_sb", tag="mk_sb")
            kT = [pp.tile([128, T], BF16, name=f"kT{g}", tag=f"kT{g}") for g in range(2)]
            qT = [pp.tile([128, T], BF16, name=f"qT{g}", tag=f"qT{g}") for g in range(2)]
            vT = [pp.tile([128, T], BF16, name=f"vT{g}", tag=f"vT{g}") for g in range(2)]
            ktok = [pp.tile([128, 16, 128], BF16, name=f"ktok{g}", tag=f"ktok{g}") for g in range(2)]
            vtok = [pp.tile([128, 16, 128], BF16, name=f"vtok{g}", tag=f"vtok{g}") for g in range(2)]
            oT = [pp.tile([128, T], BF16, name=f"oT{g}", tag=f"oT{g}") for g in range(2)]
            spad = [pp.tile([128, 128], BF16, name=f"spad{h}", tag=f"spad{h}") for h in range(4)]

            # ---- input DMAs ------------------------------------------------
            # gpsimd queue: per-d-chunk w1 tiles first (wave A starts after
            # just w1d[0] + xt[0]'s first half), then the small tensors.
            for i in range(ND):
                nc.gpsimd.dma_start(
                    out=w1d[i][:], in_=w1[128 * i:128 * (i + 1), :])
            nc.gpsimd.dma_start(out=mk_sb[:], in_=masks)
            nc.gpsimd.dma_start(out=b1_sb[:], in_=b1p)
            nc.gpsimd.dma_start(
                out=w2_sb[:], in_=w2.rearrange("(c p) f -> p c f", p=128))
            # sync queue: x^T chunk halves, first halves first so wave A can
            # start as soon as w1d[0] + xt[0] land.
            HT = T // 2
            for i in range(ND):
                nc.sync.dma_start(out=xt[i][:, 0:HT], in_=xT[128 * i:128 * (i + 1), 0:HT])
            for i in range(ND):
                nc.sync.dma_start(out=xt[i][:, HT:T], in_=xT[128 * i:128 * (i + 1), HT:T])

            # zero-fills via memset (no DMA traffic): state tiles
            for h in range(4):
                nc.vector.memset(spad[h][:], 0)

            # PE warm-up: ~3.5us of dummy matmuls during the initial DMA wait
            # so the tensor engine is at full clock when wave A starts.
            wu_w = pp.tile([128, 128], BF16, name="wu_w", tag="wu_w")
            wu_x = pp.tile([128, 512], BF16, name="wu_x", tag="wu_x")
            nc.vector.memset(wu_w[:], 0)
            nc.vector.memset(wu_x[:], 0)
            wu_p = psO.tile([128, 512], F32, name="wu_p", tag="po")
            for _ in range(16):
                nc.tensor.matmul(wu_p[:], wu_w[:], wu_x[:],
                                 start=True, stop=True, skip_group_check=True)

            # ---- phase B: projection waves ---------------------------------
            # f-tile order in w1 packing: k0 k1 v0 v1 q0 q1
            _pools = [(psA, "pa"), (psU, "pu"), (psO, "po"),
                      (psA, "pa"), (psU, "pu"), (psO, "po"),
                      (psA, "pa"), (psA, "pa")]

            def run_wave(groups):
                # groups: list of (ft, qt, copyback_dst_tile)
                tiles = []
                for gi, _ in enumerate(groups):
                    pool, tag = _pools[gi]
                    tiles.append(pool.tile([128, 512], F32, name=f"pw{gi}", tag=tag))
                for d in range(ND):
                    for gi, (ft, qt, dst) in enumerate(groups):
                        nc.tensor.matmul(
                            tiles[gi][:],
                            w1d[d][:, ft * 128:(ft + 1) * 128],
                            xt[d][:, qt * 512:(qt + 1) * 512],
                            start=(d == 0), stop=(d == ND - 1))
                        if d == ND - 1:
                            # copyback immediately after this group's last
                            # accumulation so the ACT stream starts early
                            nc.scalar.activation(
                                dst[:, qt * 512:(qt + 1) * 512], tiles[gi][:],
                                AF.Identity, bias=b1_sb[:, ft:ft + 1])

            run_wave([(0, 0, kT[0]), (1, 0, kT[1]), (2, 0, vT[0]), (3, 0, vT[1]),
                      (0, 1, kT[0]), (1, 1, kT[1]), (2, 1, vT[0]), (3, 1, vT[1])])
            run_wave([(0, 2, kT[0]), (1, 2, kT[1]), (2, 2, vT[0]), (3, 2, vT[1]),
                      (0, 3, kT[0]), (1, 3, kT[1]), (2, 3, vT[0]), (3, 3, vT[1])])
            # token-major K/V via XBAR dma transpose. All four go on ONE
            # engine queue: concurrent XBAR transposes on two queues were
            # observed to corrupt the first chunks of the second transfer.
            # They live on sync (idle by now) so the scalar engine's ACT
            # copyback stream is not delayed.
            nc.sync.dma_start_transpose(out=vtok[0][:], in_=vT[0][:])
            nc.sync.dma_start_transpose(out=vtok[1][:], in_=vT[1][:])
            nc.sync.dma_start_transpose(out=ktok[0][:], in_=kT[0][:])
            nc.sync.dma_start_transpose(out=ktok[1][:], in_=kT[1][:])
            run_wave([(4, 0, qT[0]), (5, 0, qT[1]), (4, 1, qT[0]), (5, 1, qT[1]),
                      (4, 2, qT[0]), (5, 2, qT[1]), (4, 3, qT[0]), (5, 3, qT[1])])

            # ---- phase C: chunked causal attention + interleaved D ---------
            # Two-stage software pipeline: block m's scores are emitted
            # before block m-1's O-accumulation chains, so the in-order PE
            # stream always has independent matmuls while the DVE applies
            # causal masks for the previous block.
            ablk = {}

            def scores_block(m):
                qsl = slice(m * BLK, (m + 1) * BLK)
                for pg in range(2):
                    a0 = wp.tile([128, 2 * BLK], BF16, name="a0", tag="a0", bufs=6)
                    # a1 packed [128, 256]: par's valid (below-diagonal) half
                    # of the chunk1 scores at cols par*128
                    a1 = wp.tile([128, BLK], BF16, name="a1", tag="a1", bufs=6)
                    ablk[(m, pg)] = (a0, a1)
                    for par in range(2):
                        rows = slice(par * 64, (par + 1) * 64)
                        asl = slice(par * BLK, (par + 1) * BLK)
                        pA = psA.tile([128, 2 * BLK], F32, name="pA", tag="pa")
                        nc.tensor.matmul(
                            pA[:, 0:BLK],
                            kT[pg][rows, (2 * m) * 128:(2 * m + 1) * 128],
                            qT[pg][rows, qsl], start=True, stop=True)
                        # chunk1 scores: only q-cols 128:256 of the window are
                        # below the diagonal; compute just those (ap=128).
                        nc.tensor.matmul(
                            pA[:, BLK + 128:2 * BLK],
                            kT[pg][rows, (2 * m + 1) * 128:(2 * m + 2) * 128],
                            qT[pg][rows, m * BLK + 128:(m + 1) * BLK],
                            start=True, stop=True, skip_group_check=True)
                        nc.vector.tensor_tensor(
                            a0[:, asl], pA[:, 0:BLK], mk_sb[:, 0:BLK],
                            mybir.AluOpType.mult)
                        nc.vector.tensor_tensor(
                            a1[:, par * 128:(par + 1) * 128],
                            pA[:, BLK + 128:2 * BLK], mk_sb[:, BLK + 128:2 * BLK],
                            mybir.AluOpType.mult)

            def chains_block(m):
                qsl = slice(m * BLK, (m + 1) * BLK)
                for pg in range(2):
                    a0, a1 = ablk.pop((m, pg))
                    pO = psO.tile([128, 2 * BLK], F32, name="pO", tag="po")
                    pO3 = pO.rearrange("p (c b) -> p c b", b=128)
                    nc.tensor.matmul(
                        pO[:], vtok[pg][:, 2 * m, :], a0[:],
                        start=True, stop=False)
                    # packed a1 lands on the q-cols 128:256 of each par's
                    # window: pO col-blocks 1 and 3 (strided 3D psum out)
                    nc.tensor.matmul(
                        pO3[:, 1::2, :], vtok[pg][:, 2 * m + 1, :], a1[:],
                        start=False, stop=(m == 0), skip_group_check=True)
                    if m > 0:
                        nc.tensor.matmul(
                            pO[:, 0:BLK], spad[2 * pg][:], qT[pg][:, qsl],
                            start=False, stop=False)
                        nc.tensor.matmul(
                            pO[:, BLK:2 * BLK], spad[2 * pg + 1][:], qT[pg][:, qsl],
                            start=False, stop=True)
                    for par in range(2):
                        hr = slice(par * 64, (par + 1) * 64)
                        nc.scalar.activation(
                            oT[pg][hr, qsl],
                            pO[hr, par * BLK:par * BLK + BLK], AF.Identity)

                for pg in range(2):
                    pU = psU.tile([128, 128], F32, name="pU", tag="pu")
                    nc.tensor.matmul(
                        pU[:], ktok[pg][:, 2 * m, :], vtok[pg][:, 2 * m, :],
                        start=True, stop=False)
                    nc.tensor.matmul(
                        pU[:], ktok[pg][:, 2 * m + 1, :], vtok[pg][:, 2 * m + 1, :],
                        start=False, stop=True)
                    for par in range(2):
                        h = 2 * pg + par
                        hr = slice(par * 64, (par + 1) * 64)
                        nc.vector.tensor_tensor(
                            spad[h][hr, hr], pU[hr, hr],
                            spad[h][hr, hr], mybir.AluOpType.add)

            # output staging: the 4 dout-chunks of a (qt, half) accumulate in
            # one [128, 4, 512] tile, shipped with a single DMA per half so
            # the transfer overlaps the other half's compute.
            def proj_tile(qt, half):
                fso = wp.tile([128, 4, 512], BF16, name="fso", tag="fso", bufs=2)
                dcr = range(0, ND // 2) if half == 0 else range(ND // 2, ND)
                for dc in dcr:
                    pf = [psA.tile([128, 512], F32, name="pf", tag="pa"),
                          psU.tile([128, 512], F32, name="pf2", tag="pu"),
                          psO.tile([128, 512], F32, name="pf3", tag="po")][dc % 3]
                    for g2 in range(2):
                        nc.tensor.matmul(
                            pf[:],
                            w2_sb[:, g2, dc * 128:(dc + 1) * 128],
                            oT[g2][:, qt * 512:(qt + 1) * 512],
                            start=(g2 == 0), stop=(g2 == 1))
                    if dc % 2 == 0:
                        nc.vector.tensor_copy(fso[:, dc % 4, :], pf[:])
                    else:
                        nc.scalar.activation(fso[:, dc % 4, :], pf[:], AF.Identity)
                dma_eng = nc.gpsimd if (2 * qt + half) % 2 == 0 else nc.sync
                dma_eng.dma_start(
                    out=out3[:, half * 4:(half + 1) * 4, qt * 512:(qt + 1) * 512],
                    in_=fso[:])

            # final 512-token window in 256-token pieces: block 6's piece can
            # run before block 7's chains, halving the post-chains tail
            def proj_blk(blk):
                fso = wp.tile([128, ND, BLK], BF16, name="fs2", tag="fs2", bufs=2)
                for dc in range(ND):
                    pf = [psA.tile([128, BLK], F32, name="pg1", tag="pa"),
                          psU.tile([128, BLK], F32, name="pg2", tag="pu"),
                          psO.tile([128, BLK], F32, name="pg3", tag="po")][dc % 3]
                    for g2 in range(2):
                        nc.tensor.matmul(
                            pf[:],
                            w2_sb[:, g2, dc * 128:(dc + 1) * 128],
                            oT[g2][:, blk * BLK:(blk + 1) * BLK],
                            start=(g2 == 0), stop=(g2 == 1))
                    if dc % 2 == 0:
                        nc.vector.tensor_copy(fso[:, dc, :], pf[:])
                    else:
                        nc.scalar.activation(fso[:, dc, :], pf[:], AF.Identity)
                dma_eng = nc.gpsimd if blk % 2 == 0 else nc.sync
                dma_eng.dma_start(
                    out=out3[:, :, blk * BLK:(blk + 1) * BLK], in_=fso[:])

            # proj_tile(qt) is emitted one-plus blocks after the chains that
            # produce its oT inputs, so the PE never waits on the Scalar
            # engine's oT copybacks.
            dplan = {3: [(0, 0)], 4: [(0, 1)], 5: [(1, 0)], 6: [(1, 1)],
                     7: [(2, 0), (2, 1)]}
            scores_block(0)
            for m in range(1, NBLK):
                scores_block(m)
                chains_block(m - 1)
                for pt in dplan.get(m, []):
                    proj_tile(*pt)
                if m == NBLK - 1:
                    proj_blk(6)
            chains_block(NBLK - 1)
            proj_blk(7)

    nc.compile()
    return nc


_NC = None


def _get_nc():
    global _NC
    if _NC is None:
        _NC = _build()
    return _NC


def make_core_inputs(x, W1, b1, W2, b2):
    """Shard full inputs into the 8 per-core input dicts."""
    import ml_dtypes
    BF = ml_dtypes.bfloat16
    x = np.asarray(x, dtype=np.float32)
    W1 = np.asarray(W1, dtype=np.float32).astype(BF)
    b1 = np.asarray(b1, dtype=np.float32)
    W2 = np.asarray(W2, dtype=np.float32)

    p = np.arange(128)[:, None]
    f = np.arange(BLK)[None, :]
    mask0 = (f >= p).astype(np.float32)
    mask1 = (f >= p + 128).astype(np.float32)
    masks = np.concatenate([mask0, mask1], axis=1)

    in_maps = []
    for c in range(8):
        b = c // 4
        g = c % 4
        ksl = slice(g * FH, (g + 1) * FH)
        qsl = slice(D + g * FH, D + (g + 1) * FH)
        vsl = slice(2 * D + g * FH, 2 * D + (g + 1) * FH)
        # w1 packed K | V | Q along features (matches f-tile order k0 k1 v0 v1 q0 q1)
        w1p = np.concatenate([W1[:, ksl], W1[:, vsl], W1[:, qsl]], axis=1)
        b1loc = np.concatenate([b1[ksl], b1[vsl], b1[qsl]])
        b1p = np.ascontiguousarray(b1loc.reshape(6, 128).T.astype(np.float32))
        in_maps.append({
            "xT": np.ascontiguousarray(x[b].T.astype(BF)),
            "w1": np.ascontiguousarray(w1p),
            "w2": np.ascontiguousarray(W2[ksl, :].astype(BF)),
            "b1p": b1p,
            "masks": masks,
        })
    return in_maps


def kernel(x, W1, b1, W2, b2):
    nc = _get_nc()
    in_maps = make_core_inputs(x, W1, b1, W2, b2)
    kwargs = {}
    if TRACE:
        kwargs = {"trace": True, "tmpdir": TRACE_DIR}
    res = run_bass_kernel_spmd(nc, in_maps, list(range(8)), **kwargs)
    LAST_RESULTS[0] = res
    b2 = np.asarray(b2, dtype=np.float32)
    out = np.zeros((B, T, D), np.float32)
    for c in range(8):
        oc = np.asarray(res.results[c]["out"]).astype(np.float32)
        oc = oc.reshape(128, ND, T).transpose(1, 0, 2).reshape(D, T)
        out[c // 4] += oc.T
    out += b2[None, None, :]
    return out
